# revision 1
# baseline (speedup 1.0000x reference)
"""Sliding-window causal self-attention with RoPE on 8 Trainium2 NeuronCores.

Problem: B=2, S=2048, D=1024, H=16, HD=64, WINDOW=256, fp32.
Sharding: 2 (batch) x 4 (head-groups of 4 heads). Each core computes its
head-group's QKV projections, RoPE, windowed attention, and a partial output
projection (y_g @ Wo_g.T); the host sums the 4 partials per batch.

Layout strategy: everything feature-major ("T layout").
  - xT [D, S] per batch; q/k projections emit qT/kT [dout, S] directly.
  - RoPE: qrotT = P @ qT via a tiny matmul (P = pair-swap +/- permutation),
    then q' = qT*cos + qrotT*sin on the vector engine.
  - scores^T [k, q] per 128-key block against a 384-query window (the
    128-aligned sliding-window halo); additive -240 mask pre-exp.
  - attn^T @ ... : AV as lhsT=v_aug [128,65] (ones column -> denominator),
    rhs = attnT slices, accumulated per query block in PSUM.
  - normalization: reciprocal of the denominator row, broadcast across 64
    partitions with a ones[1,64] matmul, one multiply.
  - Wo: lhsT = yT (already transposed), rhs = WoT slices.
Matmuls run in float32r (~1.5e-4 rel err, 4x faster than fp32 at N>=256).
"""
import sys

for _p in ("/opt/trn_rl_repo", "/root/.axon_site/_ro/trn_rl_repo"):
    if _p not in sys.path:
        sys.path.append(_p)

import numpy as np
import concourse.bacc as bacc
import concourse.mybir as mybir
from concourse.tile import TileContext
from concourse.bass_utils import run_bass_kernel_spmd

F32 = mybir.dt.float32
F32R = mybir.dt.float32r
AF = mybir.ActivationFunctionType

B, S, D = 2, 2048, 1024
H, HD = 16, 64
WINDOW = 256
THETA = 10000.0
SCALING = 1.0

HG = 4                      # head-groups (cores per batch)
HPG = H // HG               # heads per group = 4
GD = HPG * HD               # group out width = 256
NKB = S // 128              # 16 key blocks
NSL = 4                     # 512-wide s-chunks
SCALE = 1.0 / float(np.sqrt(HD))
MASKVAL = -240.0

_CACHE = {}
DEBUG = False


def _build():
    nc = bacc.Bacc(target_bir_lowering=False, trn_type="TRN2")

    xT = nc.dram_tensor("xT", [D, S], F32R, kind="ExternalInput")
    # weights pre-swizzled on host to [128, kt*GD] partition-major layout
    wq = nc.dram_tensor("wq", [128, D // 128 * GD], F32R, kind="ExternalInput")
    wk = nc.dram_tensor("wk", [128, D // 128 * GD], F32R, kind="ExternalInput")
    wv = nc.dram_tensor("wv", [128, D // 128 * GD], F32R, kind="ExternalInput")
    wo = nc.dram_tensor("wo", [128, GD // 128 * D], F32R, kind="ExternalInput")
    cos2 = nc.dram_tensor("cos2", [128, S], F32, kind="ExternalInput")
    sin2 = nc.dram_tensor("sin2", [128, S], F32, kind="ExternalInput")
    pt2 = nc.dram_tensor("pt2", [128, 128], F32R, kind="ExternalInput")
    mask = nc.dram_tensor("mask", [128, 384], F32, kind="ExternalInput")
    ones64 = nc.dram_tensor("ones64", [128, HPG * NKB], F32R, kind="ExternalInput")
    onesc = nc.dram_tensor("onesc", [1, 64], F32R, kind="ExternalInput")
    out = nc.dram_tensor("out", [S, D], F32, kind="ExternalOutput")
    if DEBUG:
        d_qraw = nc.dram_tensor("d_qraw", [128, S], F32, kind="ExternalOutput")
        d_qf = nc.dram_tensor("d_qf", [128, S], F32, kind="ExternalOutput")
        d_kf = nc.dram_tensor("d_kf", [128, S], F32, kind="ExternalOutput")
        d_v = nc.dram_tensor("d_v", [128, NKB * HPG * 65], F32, kind="ExternalOutput")
        d_attn = nc.dram_tensor("d_attn", [128, NKB * 384], F32, kind="ExternalOutput")
        d_yT = nc.dram_tensor("d_yT", [128, S], F32, kind="ExternalOutput")
        d_rc = nc.dram_tensor("d_rc", [1, 512], F32, kind="ExternalOutput")
        d_rbs = nc.dram_tensor("d_rbs", [64, 512], F32, kind="ExternalOutput")
        d_den = nc.dram_tensor("d_den", [1, 512], F32, kind="ExternalOutput")

    with TileContext(nc) as tc:
        with tc.tile_pool(name="const", bufs=1) as cpool, \
             tc.tile_pool(name="persist", bufs=1) as ppool:
            # resident weights/constants
            wq_sb = cpool.tile([128, D // 128, GD], F32R)
            wk_sb = cpool.tile([128, D // 128, GD], F32R)
            wv_sb = cpool.tile([128, D // 128, GD], F32R)
            wo_sb = cpool.tile([128, GD // 128, D], F32R)
            cos_sb = cpool.tile([128, S], F32)
            sin_sb = cpool.tile([128, S], F32)
            pt2_sb = cpool.tile([128, 128], F32R)
            mask_sb = cpool.tile([128, 384], F32)
            onesc_sb = cpool.tile([1, 64], F32R)
            nc.scalar.dma_start(wq_sb[:].rearrange("p a b -> p (a b)"), wq.ap())
            nc.scalar.dma_start(wk_sb[:].rearrange("p a b -> p (a b)"), wk.ap())
            nc.scalar.dma_start(pt2_sb[:], pt2[:])
            nc.scalar.dma_start(cos_sb[:], cos2[:])
            nc.scalar.dma_start(sin_sb[:], sin2[:])
            nc.scalar.dma_start(mask_sb[:], mask[:])
            nc.scalar.dma_start(onesc_sb[:], onesc[:])

            # persistent activations
            # v_sb: per key-block groups of (64 v-cols + ones col) per head
            v_sb = ppool.tile([128, NKB * HPG * 65], F32R)       # 16.25KB/part

            qf = [ppool.tile([128, S], F32R, name=f"qf{t}") for t in range(2)]
            kf = [ppool.tile([128, S], F32R, name=f"kf{t}") for t in range(2)]
            yT = [ppool.tile([128, S], F32R, name=f"yT{t}") for t in range(2)]

            # ---------------- phase 1: projections + rope ----------------
            # Weight-stationary over S-halves: one W tile serves both 512-wide
            # s-chunks of a half (amortizes LDWEIGHTS); xT rows for the half
            # stay resident; rope runs inline per evacuated chunk.
            HS = 1024
            with tc.tile_pool(name="p1x", bufs=2) as xpool, \
                 tc.tile_pool(name="p1raw", bufs=3) as rawpool, \
                 tc.tile_pool(name="p1tmp", bufs=3) as tpool, \
                 tc.tile_pool(name="p1ps", bufs=1, space="PSUM") as ps1, \
                 tc.tile_pool(name="ropeps", bufs=2, space="PSUM") as ps15:
                wsel = [(wq_sb, 0, qf[0]), (wq_sb, 128, qf[1]),
                        (wk_sb, 0, kf[0]), (wk_sb, 128, kf[1])]
                xrow = [[None] * (D // 128) for _ in range(2)]
                for half in range(2):
                    h0 = half * HS
                    for t in range(4):
                        w_t, off, dst = wsel[t]
                        acc = [ps1.tile([128, 512], F32, name=f"acc{half}_{t}_{sl}",
                                        tag=f"acc{sl}", bufs=2) for sl in range(2)]
                        for kt in range(D // 128):
                            if t == 0 and half == 0:
                                xrow[0][kt] = xpool.tile(
                                    [128, HS], F32R, tag=f"x{kt}",
                                    name=f"xrow0_{kt}")
                                nc.sync.dma_start(
                                    xrow[0][kt][:],
                                    xT.ap()[kt * 128:(kt + 1) * 128, 0:HS])
                            st, sp = (kt == 0), (kt == D // 128 - 1)
                            for sl in range(2):
                                nc.tensor.matmul(
                                    acc[sl][:], w_t[:, kt, off:off + 128],
                                    xrow[half][kt][:, sl * 512:(sl + 1) * 512],
                                    start=st, stop=sp)
                        for sl in range(2):
                            s0 = h0 + sl * 512
                            raw = rawpool.tile([128, 512], F32R, tag="raw")
                            nc.scalar.copy(raw[:], acc[sl][:])
                            rot = ps15.tile([128, 512], F32, tag="rot")
                            nc.tensor.matmul(rot[:], pt2_sb[:], raw[:],
                                             start=True, stop=True)
                            t1 = tpool.tile([128, 512], F32, tag="t1")
                            nc.vector.tensor_mul(t1[:], rot[:],
                                                 sin_sb[:, s0:s0 + 512])
                            t2 = tpool.tile([128, 512], F32, tag="t2")
                            nc.vector.tensor_mul(t2[:], raw[:].bitcast(F32),
                                                 cos_sb[:, s0:s0 + 512])
                            nc.vector.tensor_add(dst[:, s0:s0 + 512], t1[:], t2[:])
                    if half == 0:
                        for kt in range(D // 128):
                            xrow[1][kt] = xpool.tile(
                                [128, HS], F32R, tag=f"x{kt}",
                                name=f"xrow1_{kt}")
                            nc.sync.dma_start(
                                xrow[1][kt][:],
                                xT.ap()[kt * 128:(kt + 1) * 128, HS:2 * HS])
                        nc.scalar.dma_start(
                            wv_sb[:].rearrange("p a b -> p (a b)"), wv.ap())
                        nc.scalar.dma_start(
                            v_sb[:].rearrange("p (g c) -> p g c", c=65)[:, :, 64],
                            ones64[:])
                    # v projection sweep for this half (xT rows resident).
                    # start only on the bank first matmul: start=True clears
                    # has_written for the WHOLE bank.
                    for g in range(2):
                        vacc = [ps1.tile([128, 512], F32, name=f"vacc{half}_{g}_{j}",
                                         tag=f"vacc{j}") for j in range(2)]
                        for kt in range(D // 128):
                            st, sp = (kt == 0), (kt == D // 128 - 1)
                            for j in range(2):
                                for jj in range(2):
                                    stl = g * 4 + 2 * j + jj
                                    nc.tensor.matmul(
                                        vacc[j][:, jj * 256:(jj + 1) * 256],
                                        xrow[half][kt][:, stl * 128:(stl + 1) * 128],
                                        wv_sb[:, kt, 0:256],
                                        start=(st and jj == 0), stop=sp)
                        for j in range(2):
                            for jj in range(2):
                                kb = half * 8 + g * 4 + 2 * j + jj
                                dstv = v_sb[:, kb * HPG * 65:(kb + 1) * HPG * 65]
                                nc.scalar.copy(
                                    dstv.rearrange("p (g c) -> p g c", c=65)[:, :, 0:64],
                                    vacc[j][:, jj * 256:(jj + 1) * 256]
                                    .rearrange("p (g c) -> p g c", c=64))

            if DEBUG:
                nc.sync.dma_start(d_qraw[:], qf[0][:].bitcast(F32))

            if DEBUG:
                nc.sync.dma_start(d_qf[:], qf[0][:].bitcast(F32))
                nc.sync.dma_start(d_kf[:], kf[0][:].bitcast(F32))
                nc.sync.dma_start(d_v[:], v_sb[:].bitcast(F32))

            # ---------------- phase 2: attention ----------------
            # Head-PAIR processing: the two heads of a pair live on partition
            # halves 0:64 / 64:128, so their K=64 scores matmuls load into
            # disjoint PE row-groups and overlap in the array when adjacent.
            with tc.tile_pool(name="attn", bufs=1) as apool, \
                 tc.tile_pool(name="smalls", bufs=4) as spool, \
                 tc.tile_pool(name="scps", bufs=6, space="PSUM") as scps, \
                 tc.tile_pool(name="avps", bufs=2, space="PSUM") as avps:
                for th in range(2):
                    attns = [apool.tile([128, NKB * 384], F32R,
                                        name=f"attn{th}_{i}", tag=f"attn{i}")
                             for i in range(2)]
                    if DEBUG and th == 1:
                        nc.sync.dma_start(d_attn[:], attns0_dbg[:].bitcast(F32))
                    attns0_dbg = attns[0]
                    for kb in range(NKB):
                        q0 = kb * 128
                        n = min(384, S - q0)
                        for i in range(2):
                            ph = 64 * i
                            sc = scps.tile([128, 384], F32, tag="sc",
                                           name=f"sc{th}_{kb}_{i}")
                            nc.tensor.matmul(sc[:, 0:n],
                                             kf[th][ph:ph + 64, q0:q0 + 128],
                                             qf[th][ph:ph + 64, q0:q0 + n],
                                             start=True, stop=True)
                            if n == 384:
                                scv = sc[:].rearrange("p (g c) -> p g c", g=3)[:, 0::2, :]
                                mkv = mask_sb[:].rearrange("p (g c) -> p g c", g=3)[:, 0::2, :]
                                nc.vector.tensor_add(scv, scv, mkv)
                            else:
                                nc.vector.tensor_add(sc[:, 0:128], sc[:, 0:128],
                                                     mask_sb[:, 0:128])
                            nc.scalar.activation(attns[i][:, kb * 384:kb * 384 + n],
                                                 sc[:, 0:n], AF.Exp, scale=SCALE)
                    for i in range(2):
                        h = 2 * th + i
                        ph = 64 * i
                        attn_h = attns[i]
                        for qq in range(4):
                            acc = avps.tile([65, 512], F32, tag="av",
                                            name=f"av{th}_{i}_{qq}")
                            first = True
                            for j2 in range(2):          # qb pair (2m, 2m+1)
                                m = 2 * qq + j2
                                qb0 = 2 * m
                                mms = []
                                if m >= 1:
                                    mms.append((qb0 - 2, 0, 2 * 128, 128))
                                    mms.append((qb0 - 1, 0, 128, 256))
                                    mms.append((qb0, 0, 0, 256))
                                else:
                                    mms.append((qb0, 0, 0, 256))
                                mms.append((qb0 + 1, 128, 0, 128))
                                for ii, (kb, jo, ao, w) in enumerate(mms):
                                    wdt = min(w, S - kb * 128 - ao)
                                    nc.tensor.matmul(
                                        acc[:, j2 * 256 + jo:j2 * 256 + jo + wdt],
                                        v_sb[:, (kb * HPG + h) * 65:
                                             (kb * HPG + h) * 65 + 65],
                                        attn_h[:, kb * 384 + ao:kb * 384 + ao + wdt],
                                        start=first,
                                        stop=(j2 == 1 and ii == len(mms) - 1))
                                    first = False
                            den = spool.tile([1, 512], F32, tag="dent")
                            nc.scalar.copy(den[:], acc[64:65, :])
                            rc0 = spool.tile([1, 512], F32, tag="rc0")
                            nc.vector.reciprocal_approx_fast(out=rc0[:], in_=den[:])
                            # broadcast partition 0 across 64 partitions on GpSimd
                            rbs = spool.tile([64, 512], F32, tag="rbs")
                            nc.gpsimd.partition_broadcast(rbs[:], rc0[:])
                            if DEBUG and h == 0 and qq == 0:
                                den_sb = spool.tile([1, 512], F32, tag="den")
                                nc.scalar.copy(den_sb[:], acc[64:65, :])
                                nc.sync.dma_start(d_den[:], den_sb[:])
                                nc.sync.dma_start(d_rc[:], rc0[:])
                                nc.sync.dma_start(d_rbs[:], rbs[:])
                            nc.vector.tensor_mul(
                                yT[th][ph:ph + 64, qq * 512:(qq + 1) * 512],
                                acc[0:64, :], rbs[:])

            if DEBUG:
                nc.sync.dma_start(d_yT[:], yT[0][:].bitcast(F32))

            # ---------------- phase 3: output projection ----------------
            nc.scalar.dma_start(wo_sb[:].rearrange("p a b -> p (a b)"), wo.ap())
            with tc.tile_pool(name="p3sb", bufs=3) as opool, \
                 tc.tile_pool(name="p3ps", bufs=4, space="PSUM") as ps3:
                for stile in range(S // 128):
                    r0 = stile * 128
                    ot = opool.tile([128, D], F32, tag="ot")
                    for dc in range(2):
                        oacc = ps3.tile([128, 512], F32, tag="oacc")
                        for ct in range(2):
                            nc.tensor.matmul(oacc[:],
                                             yT[ct][:, r0:r0 + 128],
                                             wo_sb[:, ct, dc * 512:(dc + 1) * 512],
                                             start=(ct == 0), stop=(ct == 1))
                        if dc == 0:
                            nc.scalar.copy(ot[:, 0:512], oacc[:])
                        else:
                            nc.vector.tensor_copy(ot[:, 512:1024], oacc[:])
                        nc.sync.dma_start(
                            out.ap()[r0:r0 + 128, dc * 512:(dc + 1) * 512],
                            ot[:, dc * 512:(dc + 1) * 512])

    nc.finalize()
    return nc


def _rope_tables():
    inv_freq = 1.0 / (THETA ** (np.arange(0, HD, 2, dtype=np.float64) / HD))
    t = np.arange(S, dtype=np.float64) / max(SCALING, 1e-6)
    freqs = np.outer(t, inv_freq)                      # [S, HD/2]
    emb = np.concatenate((freqs, freqs), axis=-1)      # [S, HD]
    return np.cos(emb).astype(np.float32), np.sin(emb).astype(np.float32)


def _swz(w):
    # [kt*128, X] -> [128, kt*X] partition-major contiguous
    kt = w.shape[0] // 128
    return np.ascontiguousarray(
        w.reshape(kt, 128, w.shape[1]).transpose(1, 0, 2).reshape(128, -1))


def _host_prep(x, Wq, Wk, Wv, Wo):
    cos, sin = _rope_tables()
    cosT2 = np.ascontiguousarray(np.tile(cos.T, (2, 1)))     # [128, S]
    sinT2 = np.ascontiguousarray(np.tile(sin.T, (2, 1)))
    P = np.zeros((HD, HD), dtype=np.float32)
    for i in range(HD // 2):
        P[2 * i, 2 * i + 1] = -1.0
        P[2 * i + 1, 2 * i] = 1.0
    PT = P.T
    pt2 = np.zeros((128, 128), dtype=np.float32)
    pt2[0:64, 0:64] = PT
    pt2[64:128, 64:128] = PT

    ii = np.arange(384)[None, :]          # query offset within window
    jj = np.arange(128)[:, None]          # key offset within block
    m = np.zeros((128, 384), dtype=np.float32)
    m[:, 0:128] += np.where(ii[:, 0:128] >= jj, 0.0, MASKVAL)
    m[:, 256:384] += np.where(ii[:, 256:384] - 256 < jj, 0.0, MASKVAL)

    ones64 = np.ones((128, HPG * NKB), dtype=np.float32)
    onesc = np.ones((1, 64), dtype=np.float32)

    in_maps = []
    for c in range(8):
        b, g = c // HG, c % HG
        gsl = slice(g * GD, (g + 1) * GD)
        in_maps.append({
            "xT": np.ascontiguousarray(x[b].T),
            "wq": _swz(Wq[gsl, :].T),
            "wk": _swz(Wk[gsl, :].T),
            "wv": _swz(Wv[gsl, :].T),
            "wo": _swz(Wo[:, gsl].T),
            "cos2": cosT2, "sin2": sinT2, "pt2": pt2, "mask": m,
            "ones64": ones64, "onesc": onesc,
        })
    return in_maps


def _run(inputs, trace=False, **kw):
    if "nc" not in _CACHE:
        _CACHE["nc"] = _build()
    in_maps = _host_prep(inputs["x"], inputs["Wq"], inputs["Wk"],
                         inputs["Wv"], inputs["Wo"])
    return run_bass_kernel_spmd(_CACHE["nc"], in_maps, list(range(8)),
                                trace=trace, **kw)


def kernel(x, Wq, Wk, Wv, Wo):
    res = _run({"x": x, "Wq": Wq, "Wk": Wk, "Wv": Wv, "Wo": Wo})
    out = np.zeros((B, S, D), dtype=np.float32)
    for c in range(8):
        out[c // HG] += res.results[c]["out"]
    return out



# revision 11
# speedup vs baseline: 1.3661x; 1.3661x over previous
"""Sliding-window causal self-attention with RoPE on 8 Trainium2 NeuronCores.

Problem: B=2, S=2048, D=1024, H=16, HD=64, WINDOW=256, fp32.
Sharding: 2 (batch) x 4 (head-groups of 4 heads). Each core computes its
head-group's QKV projections, RoPE, windowed attention, and a partial output
projection (y_g @ Wo_g.T); the host sums the 4 partials per batch.

v2: bf16 matmul paths everywhere (error budget 2e-2 >> bf16 noise), built for
PE saturation (HAM clock ramps 1.2->2.4GHz after ~3.4us of continuous busy):
  - x streamed in 4 s-chunks of 512 tokens, host-swizzled so every DMA row is
    one contiguous 4KB descriptor; weights land in halves so matmuls start
    within a few us.
  - phase 1 per chunk: kt-outer q/k sweep (N=512 matmuls into 4 PSUM banks),
    RoPE evac (pt2 rot matmul + DVE muls; t2 on GpSimd off the DVE), then a
    v sweep with positions-on-partitions (N=256), evac to v_aug layout.
  - v_aug per (kb, head) = [64 v-cols | 64 ones-cols]: the AV matmul then
    yields y rows 0:64 and the softmax denominator replicated on rows 64:128,
    so normalization is a 64-partition reciprocal + one DVE multiply (no
    partition broadcast, no single-partition ops).
  - scores per (pair, kb, head) [128 k, 384 q] + additive -240 mask pre-exp;
    exp emits bf16 attn directly.
"""
import sys

for _p in ("/opt/trn_rl_repo", "/root/.axon_site/_ro/trn_rl_repo"):
    if _p not in sys.path:
        sys.path.append(_p)

import numpy as np
import ml_dtypes
import concourse.bacc as bacc
import concourse.mybir as mybir
from concourse.tile import TileContext
from concourse.bass_utils import run_bass_kernel_spmd

F32 = mybir.dt.float32
BF16 = mybir.dt.bfloat16
AF = mybir.ActivationFunctionType
BF = ml_dtypes.bfloat16

B, S, D = 2, 2048, 1024
H, HD = 16, 64
WINDOW = 256
THETA = 10000.0
SCALING = 1.0

HG = 4                      # head-groups (cores per batch)
HPG = H // HG               # heads per group = 4
GD = HPG * HD               # group out width = 256
NKB = S // 128              # 16 key blocks
NSC = 4                     # 512-token s-chunks
KT = D // 128               # 8 contraction chunks
SCALE = 1.0 / float(np.sqrt(HD))
MASKVAL = -240.0

_CACHE = {}
DEBUG = False


def _build():
    nc = bacc.Bacc(target_bir_lowering=False, trn_type="TRN2")

    # x swizzled [128, (schunk, kt, 512)]: row p = x[s*512+j, kt*128+p]
    xs = nc.dram_tensor("xs", [128, NSC * KT * 512], BF16, kind="ExternalInput")
    # weights pre-swizzled on host to [128, kt*cols] partition-major
    wq = nc.dram_tensor("wq", [128, KT * GD], BF16, kind="ExternalInput")
    wk = nc.dram_tensor("wk", [128, KT * GD], BF16, kind="ExternalInput")
    wv = nc.dram_tensor("wv", [128, KT * GD], BF16, kind="ExternalInput")
    wo = nc.dram_tensor("wo", [128, (GD // 128) * D], BF16, kind="ExternalInput")
    cosb = nc.dram_tensor("cosb", [128, S], BF16, kind="ExternalInput")
    sinf = nc.dram_tensor("sinf", [128, S], F32, kind="ExternalInput")
    pt2 = nc.dram_tensor("pt2", [128, 128], BF16, kind="ExternalInput")
    mask = nc.dram_tensor("mask", [128, 384], F32, kind="ExternalInput")
    out = nc.dram_tensor("out", [S, D], F32, kind="ExternalOutput")
    if DEBUG:
        d_qf = nc.dram_tensor("d_qf", [128, S], BF16, kind="ExternalOutput")
        d_kf = nc.dram_tensor("d_kf", [128, S], BF16, kind="ExternalOutput")
        d_v = nc.dram_tensor("d_v", [128, NKB * HPG * 128], BF16,
                             kind="ExternalOutput")
        d_attn = nc.dram_tensor("d_attn", [128, NKB * 384], BF16,
                                kind="ExternalOutput")
        d_yT = nc.dram_tensor("d_yT", [128, S], BF16, kind="ExternalOutput")
        d_rbs = nc.dram_tensor("d_rbs", [64, 512], F32, kind="ExternalOutput")
        d_den = nc.dram_tensor("d_den", [64, 512], F32, kind="ExternalOutput")

    with TileContext(nc) as tc:
        with tc.tile_pool(name="const", bufs=1) as cpool, \
             tc.tile_pool(name="persist", bufs=1) as ppool:
            wq_sb = cpool.tile([128, KT, GD], BF16)
            wk_sb = cpool.tile([128, KT, GD], BF16)
            wv_sb = cpool.tile([128, KT, GD], BF16)
            wo_sb = cpool.tile([128, GD // 128, D], BF16)
            cosb_sb = cpool.tile([128, S], BF16)
            sin_sb = cpool.tile([128, S], F32)
            pt2_sb = cpool.tile([128, 128], BF16)
            mask_sb = cpool.tile([128, 384], F32)

            # weight lo/hi halves on separate queues so kt=0 work starts early
            for w_dram, w_sb, eng in ((wq, wq_sb, nc.scalar),
                                      (wk, wk_sb, nc.scalar),
                                      (wv, wv_sb, nc.gpsimd)):
                wv2 = w_sb[:].rearrange("p a b -> p (a b)")
                half = KT // 2 * GD
                eng.dma_start(wv2[:, 0:half], w_dram.ap()[:, 0:half])
                eng.dma_start(wv2[:, half:2 * half], w_dram.ap()[:, half:2 * half])
            nc.scalar.dma_start(pt2_sb[:], pt2[:])
            nc.gpsimd.dma_start(cosb_sb[:], cosb[:])
            nc.gpsimd.dma_start(sin_sb[:], sinf[:])
            nc.gpsimd.dma_start(mask_sb[:], mask[:])
            nc.scalar.dma_start(wo_sb[:].rearrange("p a b -> p (a b)"), wo.ap())

            # x chunks: halves (kt 0-3 / 4-7) so the first sweep starts early
            xs_sb = []
            for s in range(NSC):
                t = ppool.tile([128, KT, 512], BF16, name=f"xs{s}")
                xs_sb.append(t)
                tv = t[:].rearrange("p a b -> p (a b)")
                c0 = s * KT * 512
                nc.sync.dma_start(tv[:, 0:KT * 256],
                                  xs.ap()[:, c0:c0 + KT * 256])
                nc.sync.dma_start(tv[:, KT * 256:KT * 512],
                                  xs.ap()[:, c0 + KT * 256:c0 + KT * 512])

            qf = [ppool.tile([128, S], BF16, name=f"qf{t}") for t in range(2)]
            kf = [ppool.tile([128, S], BF16, name=f"kf{t}") for t in range(2)]
            yT = [ppool.tile([128, S], BF16, name=f"yT{t}") for t in range(2)]
            # v_aug: per (kb, h) 128 cols = [64 v | 64 ones]
            v_sb = ppool.tile([128, NKB * HPG * 128], BF16)
            ones_view = v_sb[:].rearrange("p (k c) -> p k c", c=128)[:, :, 64:128]
            nc.gpsimd.memset(ones_view, 1.0)

            # ---------------- phase 1: projections + rope ----------------
            with tc.tile_pool(name="p1raw", bufs=3) as rawp, \
                 tc.tile_pool(name="p1t", bufs=4) as tp, \
                 tc.tile_pool(name="p1ps", bufs=1, space="PSUM") as ps1, \
                 tc.tile_pool(name="rotps", bufs=2, space="PSUM") as rotps:
                wsel = [(wq_sb, 0, qf[0]), (wq_sb, 128, qf[1]),
                        (wk_sb, 0, kf[0]), (wk_sb, 128, kf[1])]
                for s in range(NSC):
                    s0 = s * 512
                    # q/k sweep, kt-outer (4 live accumulators)
                    accs = [ps1.tile([128, 512], F32, tag=f"a{t}",
                                     name=f"acc{s}_{t}") for t in range(4)]
                    for kt in range(KT):
                        st, sp = (kt == 0), (kt == KT - 1)
                        for t, (w_t, off, _) in enumerate(wsel):
                            nc.tensor.matmul(accs[t][:],
                                             w_t[:, kt, off:off + 128],
                                             xs_sb[s][:, kt, :],
                                             start=st, stop=sp)
                    # rope evac per target
                    for t, (_, _, dstf) in enumerate(wsel):
                        raw = rawp.tile([128, 512], BF16, tag="raw")
                        nc.scalar.copy(raw[:], accs[t][:])
                        rot = rotps.tile([128, 512], F32, tag="rot",
                                         name=f"rot{s}_{t}")
                        nc.tensor.matmul(rot[:], pt2_sb[:], raw[:],
                                         start=True, stop=True)
                        t2 = tp.tile([128, 512], BF16, tag="t2")
                        nc.gpsimd.tensor_mul(t2[:], raw[:],
                                             cosb_sb[:, s0:s0 + 512])
                        t1 = tp.tile([128, 512], BF16, tag="t1")
                        nc.vector.tensor_mul(t1[:], rot[:],
                                             sin_sb[:, s0:s0 + 512])
                        nc.vector.tensor_add(dstf[:, s0:s0 + 512], t1[:], t2[:])
                    # v sweep: out [128 pos, 256 feat] per pos-block
                    vaccs = [ps1.tile([128, 512], F32, tag=f"v{j}",
                                      name=f"vacc{s}_{j}") for j in range(2)]
                    for kt in range(KT):
                        st, sp = (kt == 0), (kt == KT - 1)
                        for j in range(2):
                            for jj in range(2):
                                pb = 2 * j + jj
                                nc.tensor.matmul(
                                    vaccs[j][:, jj * 256:(jj + 1) * 256],
                                    xs_sb[s][:, kt, pb * 128:(pb + 1) * 128],
                                    wv_sb[:, kt, 0:256],
                                    start=(st and jj == 0), stop=sp)
                    for j in range(2):
                        for jj in range(2):
                            kb = 4 * s + 2 * j + jj
                            dstv = v_sb[:, kb * HPG * 128:(kb + 1) * HPG * 128] \
                                .rearrange("p (h c) -> p h c", c=128)[:, :, 0:64]
                            nc.scalar.copy(
                                dstv,
                                vaccs[j][:, jj * 256:(jj + 1) * 256]
                                .rearrange("p (h c) -> p h c", c=64))

            if DEBUG:
                nc.sync.dma_start(d_qf[:], qf[0][:])
                nc.sync.dma_start(d_kf[:], kf[0][:])
                nc.sync.dma_start(d_v[:], v_sb[:])

            # ---------------- phase 2: attention ----------------
            with tc.tile_pool(name="attn", bufs=1) as apool, \
                 tc.tile_pool(name="rbsp", bufs=4) as rbsp, \
                 tc.tile_pool(name="scps", bufs=4, space="PSUM") as scps, \
                 tc.tile_pool(name="avps", bufs=4, space="PSUM") as avps:
                attns = [apool.tile([128, NKB * 384], BF16, name=f"attn{h}",
                                    tag=f"attn{h}") for h in range(4)]
                for th in range(2):
                    for kb in range(NKB):
                        q0 = kb * 128
                        n = min(384, S - q0)
                        for i in range(2):
                            ph = 64 * i
                            sc = scps.tile([128, 384], F32, tag="sc",
                                           name=f"sc{th}_{kb}_{i}")
                            nc.tensor.matmul(sc[:, 0:n],
                                             kf[th][ph:ph + 64, q0:q0 + 128],
                                             qf[th][ph:ph + 64, q0:q0 + n],
                                             start=True, stop=True)
                            if n == 384:
                                scv = sc[:].rearrange("p (g c) -> p g c", g=3)[:, 0::2, :]
                                mkv = mask_sb[:].rearrange("p (g c) -> p g c", g=3)[:, 0::2, :]
                                nc.vector.tensor_add(scv, scv, mkv)
                            else:
                                nc.vector.tensor_add(sc[:, 0:128], sc[:, 0:128],
                                                     mask_sb[:, 0:128])
                            nc.scalar.activation(
                                attns[2 * th + i][:, kb * 384:kb * 384 + n],
                                sc[:, 0:n], AF.Exp, scale=SCALE)
                for th in range(2):
                    for i in range(2):
                        h = 2 * th + i
                        attn_h = attns[h]
                        for qq in range(4):
                            acc = avps.tile([128, 512], F32, tag="av",
                                            name=f"av{h}_{qq}")
                            first = True
                            for j2 in range(2):          # qb pair (2m, 2m+1)
                                m = 2 * qq + j2
                                qb0 = 2 * m
                                mms = []
                                if m >= 1:
                                    mms.append((qb0 - 2, 0, 2 * 128, 128))
                                    mms.append((qb0 - 1, 0, 128, 256))
                                    mms.append((qb0, 0, 0, 256))
                                else:
                                    mms.append((qb0, 0, 0, 256))
                                mms.append((qb0 + 1, 128, 0, 128))
                                for ii, (kb, jo, ao, w) in enumerate(mms):
                                    wdt = min(w, S - kb * 128 - ao)
                                    vcol = (kb * HPG + h) * 128
                                    nc.tensor.matmul(
                                        acc[:, j2 * 256 + jo:j2 * 256 + jo + wdt],
                                        v_sb[:, vcol:vcol + 128],
                                        attn_h[:, kb * 384 + ao:kb * 384 + ao + wdt],
                                        start=first,
                                        stop=(j2 == 1 and ii == len(mms) - 1))
                                    first = False
                            den_sb = rbsp.tile([64, 512], F32, tag="den")
                            nc.scalar.copy(den_sb[:], acc[64:128, :])
                            rbs = rbsp.tile([64, 512], F32, tag="rbs")
                            nc.vector.reciprocal_approx_fast(
                                out=rbs[:], in_=den_sb[:])
                            nc.vector.tensor_mul(
                                yT[th][64 * i:64 * i + 64, qq * 512:(qq + 1) * 512],
                                acc[0:64, :], rbs[:])
                            if DEBUG and h == 0 and qq == 0:
                                nc.sync.dma_start(d_rbs[:], rbs[:])
                                nc.sync.dma_start(d_den[:], den_sb[:])

            if DEBUG:
                nc.sync.dma_start(d_attn[:], attns[0][:])
                nc.sync.dma_start(d_yT[:], yT[0][:])

            # ---------------- phase 3: output projection ----------------
            with tc.tile_pool(name="p3sb", bufs=3) as opool, \
                 tc.tile_pool(name="p3ps", bufs=4, space="PSUM") as ps3:
                for stile in range(S // 128):
                    r0 = stile * 128
                    ot = opool.tile([128, D], F32, tag="ot")
                    for dc in range(2):
                        oacc = ps3.tile([128, 512], F32, tag="oacc")
                        for ct in range(2):
                            nc.tensor.matmul(oacc[:],
                                             yT[ct][:, r0:r0 + 128],
                                             wo_sb[:, ct, dc * 512:(dc + 1) * 512],
                                             start=(ct == 0), stop=(ct == 1))
                        if dc == 0:
                            nc.scalar.copy(ot[:, 0:512], oacc[:])
                        else:
                            nc.vector.tensor_copy(ot[:, 512:1024], oacc[:])
                    nc.sync.dma_start(out.ap()[r0:r0 + 128, :], ot[:])

    nc.finalize()
    return nc


def _rope_tables():
    inv_freq = 1.0 / (THETA ** (np.arange(0, HD, 2, dtype=np.float64) / HD))
    t = np.arange(S, dtype=np.float64) / max(SCALING, 1e-6)
    freqs = np.outer(t, inv_freq)                      # [S, HD/2]
    emb = np.concatenate((freqs, freqs), axis=-1)      # [S, HD]
    return np.cos(emb).astype(np.float32), np.sin(emb).astype(np.float32)


def _swz(w):
    # [kt*128, X] -> [128, kt*X] partition-major contiguous
    kt = w.shape[0] // 128
    return np.ascontiguousarray(
        w.reshape(kt, 128, w.shape[1]).transpose(1, 0, 2).reshape(128, -1))


def _host_prep(x, Wq, Wk, Wv, Wo):
    cos, sin = _rope_tables()
    cosT2 = np.ascontiguousarray(np.tile(cos.T, (2, 1)))     # [128, S]
    sinT2 = np.ascontiguousarray(np.tile(sin.T, (2, 1)))
    P = np.zeros((HD, HD), dtype=np.float32)
    for i in range(HD // 2):
        P[2 * i, 2 * i + 1] = -1.0
        P[2 * i + 1, 2 * i] = 1.0
    PT = P.T
    pt2 = np.zeros((128, 128), dtype=np.float32)
    pt2[0:64, 0:64] = PT
    pt2[64:128, 64:128] = PT

    ii = np.arange(384)[None, :]          # query offset within window
    jj = np.arange(128)[:, None]          # key offset within block
    m = np.zeros((128, 384), dtype=np.float32)
    m[:, 0:128] += np.where(ii[:, 0:128] >= jj, 0.0, MASKVAL)
    m[:, 256:384] += np.where(ii[:, 256:384] - 256 < jj, 0.0, MASKVAL)

    in_maps = []
    for c in range(8):
        b, g = c // HG, c % HG
        gsl = slice(g * GD, (g + 1) * GD)
        xT = x[b].T                                         # [D, S]
        # [128, (s, kt, 512)]
        xsw = np.ascontiguousarray(
            xT.reshape(KT, 128, NSC, 512).transpose(1, 2, 0, 3)
            .reshape(128, -1)).astype(BF)
        in_maps.append({
            "xs": xsw,
            "wq": _swz(Wq[gsl, :].T).astype(BF),
            "wk": _swz(Wk[gsl, :].T).astype(BF),
            "wv": _swz(Wv[gsl, :].T).astype(BF),
            "wo": _swz(Wo[:, gsl].T).astype(BF),
            "cosb": cosT2.astype(BF), "sinf": sinT2,
            "pt2": pt2.astype(BF), "mask": m,
        })
    return in_maps


def _run(inputs, trace=False, **kw):
    if "nc" not in _CACHE:
        _CACHE["nc"] = _build()
    in_maps = _host_prep(inputs["x"], inputs["Wq"], inputs["Wk"],
                         inputs["Wv"], inputs["Wo"])
    return run_bass_kernel_spmd(_CACHE["nc"], in_maps, list(range(8)),
                                trace=trace, **kw)


def kernel(x, Wq, Wk, Wv, Wo):
    res = _run({"x": x, "Wq": Wq, "Wk": Wk, "Wv": Wv, "Wo": Wo})
    out = np.zeros((B, S, D), dtype=np.float32)
    for c in range(8):
        out[c // HG] += res.results[c]["out"]
    return out
